# revision 1
# baseline (speedup 1.0000x reference)
"""Trainium2 Bass kernel for batched 3x3 VALID conv (NCHW / OIHW).

x: [32, 128, 64, 64] f32, weight: [256, 128, 3, 3] f32 -> out: [32, 256, 62, 62] f32.

Strategy: data-parallel over batch across 8 NeuronCores (4 images each).
Per core, conv is computed as 9 shift-matmuls accumulated in PSUM:
  out[co, y, x] += W[dy,dx][ci,co].T @ x[ci, y+dy, x+dx]
Groups of 8 output rows use a contiguous 512-wide moving operand (full
64-wide rows; the 2 garbage columns are stripped by the PSUM->SBUF copy).
The final 6-row group uses a strided [6, 62] window so no reads go past
the image. All matmul operands are float32r (full-rate PE mode).
"""

import numpy as np

_B, _CIN, _H, _W = 32, 128, 64, 64
_COUT = 256
_HO, _WO = 62, 62
_NCORES = 8
_BPC = _B // _NCORES  # images per core
_TAPS = 9
_GROUPS = [(r0, min(8, _HO - r0)) for r0 in range(0, _HO, 8)]

_nc_cache = None


def _build():
    global _nc_cache
    if _nc_cache is not None:
        return _nc_cache

    import concourse.bass as bass
    import concourse.mybir as mybir
    from concourse import bacc
    from concourse.tile import TileContext

    f32 = mybir.dt.float32
    f32r = mybir.dt.float32r

    nc = bacc.Bacc("TRN2", target_bir_lowering=False)
    x_d = nc.dram_tensor("x", [_BPC, _CIN, _H, _W], f32r, kind="ExternalInput")
    w_d = nc.dram_tensor("w", [_CIN, _TAPS, _COUT], f32r, kind="ExternalInput")
    o_d = nc.dram_tensor("o", [_BPC, _COUT, _HO, _WO], f32, kind="ExternalOutput")

    with TileContext(nc) as tc:
        with (
            tc.tile_pool(name="wpool", bufs=1) as wpool,
            tc.tile_pool(name="xpool", bufs=2) as xpool,
            tc.tile_pool(name="spool", bufs=4) as spool,
            tc.tile_pool(name="pspool", bufs=6, space=bass.MemorySpace.PSUM) as pspool,
        ):
            w_sb = wpool.tile([_CIN, _TAPS, _COUT], f32r)
            x_tile_a = xpool.tile([_CIN, _H, _W], f32r, tag="x")
            x_tile_b = xpool.tile([_CIN, _H, _W], f32r, tag="x")
            x_tiles = [x_tile_a, x_tile_b]

            # PE warmup: dummy matmuls on a zeroed bf16 tile while the head
            # DMAs stream in, so the HAM clock gate is ramping before the
            # real matmuls start.
            wup = wpool.tile([128, 512], mybir.dt.bfloat16)
            wps = pspool.tile([128, 512], f32, tag="wps", bufs=1)
            nc.vector.memset(wup[:], 0)
            for _ in range(16):
                nc.tensor.matmul(wps[:], wup[:, 0:128], wup[:], start=True, stop=True)

            # Head DMAs, spread over all three DMA queues in first-use order.
            # The first three PSUM groups (rows 0..25) interleave their taps
            # below to match the weight arrival order. Image prefetches are
            # queued BEHIND the weights on the scalar queue so they cannot
            # starve the critical head bytes (img2/img3 prefetches also gate
            # naturally on the WAR dependency against the x tile they reuse).
            nc.sync.dma_start(x_tiles[0][:, 0:26, :], x_d[0, :, 0:26, :])
            nc.scalar.dma_start(w_sb[:, 0:3, :], w_d[:, 0:3, :])
            nc.scalar.dma_start(w_sb[:, 3:6, :], w_d[:, 3:6, :])
            nc.scalar.dma_start(w_sb[:, 6:9, :], w_d[:, 6:9, :])
            nc.gpsimd.dma_start(x_tiles[0][:, 26:46, :], x_d[0, :, 26:46, :])
            nc.gpsimd.dma_start(x_tiles[0][:, 46:64, :], x_d[0, :, 46:64, :])

            def mm(ps, img_tiles, ct, r0, nr, tap, start, stop):
                x_sb, x_flat = img_tiles
                dy, dx = divmod(tap, 3)
                if nr == 8:
                    rhs = x_flat[:, (r0 + dy) * _W + dx : (r0 + dy) * _W + dx + nr * _W]
                else:
                    rhs = x_sb[:, r0 + dy : r0 + dy + nr, dx : dx + _WO]
                nc.tensor.matmul(
                    ps[:],
                    w_sb[:, tap, ct * 128 : (ct + 1) * 128],
                    rhs,
                    start=start,
                    stop=stop,
                )

            def finish_group(ps, img, ct, r0, nr, pipelined_tail=False):
                st = spool.tile([128, nr, _WO], f32, tag="st")
                o_slice = o_d[img, ct * 128 : (ct + 1) * 128, r0 : r0 + nr, :]
                if pipelined_tail:
                    # final group: overlap the store with the copy in two
                    # halves on the (warm) sync queue to shorten the tail
                    h = nr // 2
                    nc.vector.tensor_copy(st[:, 0:h, :], ps[:, 0:h, 0:_WO])
                    nc.sync.dma_start(o_slice[:, 0:h, :], st[:, 0:h, :])
                    nc.vector.tensor_copy(st[:, h:nr, :], ps[:, h:nr, 0:_WO])
                    nc.sync.dma_start(o_slice[:, h:nr, :], st[:, h:nr, :])
                else:
                    nc.vector.tensor_copy(st[:], ps[:, :, 0:_WO])
                    nc.sync.dma_start(o_slice, st[:])

            for img in range(_BPC):
                x_sb = x_tiles[img % 2]
                x_flat = x_sb[:].rearrange("p h w -> p (h w)")
                img_tiles = (x_sb, x_flat)
                for ct in range(_COUT // 128):
                    # Prefetch next image while the first cout-tile computes.
                    if ct == 1 and img + 1 < _BPC:
                        nxt = x_tiles[(img + 1) % 2]
                        nc.scalar.dma_start(nxt[:], x_d[img + 1])
                    if img == 0 and ct == 0:
                        # Interleave the first three groups by tap-triple so the
                        # PE consumes weight taps in the order the scalar-queue
                        # DMAs deliver them, with no stall.
                        ps0 = pspool.tile([128, 8, _W], f32, tag="ps")
                        ps1 = pspool.tile([128, 8, _W], f32, tag="ps")
                        ps2 = pspool.tile([128, 8, _W], f32, tag="ps")
                        head_ps = [ps0, ps1, ps2]
                        for t0 in range(0, _TAPS, 3):
                            for gi, ps in enumerate(head_ps):
                                for tap in range(t0, t0 + 3):
                                    mm(ps, img_tiles, 0, gi * 8, 8, tap,
                                       start=(tap == 0), stop=(tap == _TAPS - 1))
                        for gi, ps in enumerate(head_ps):
                            finish_group(ps, 0, 0, gi * 8, 8)
                        rest = _GROUPS[3:]
                    else:
                        rest = _GROUPS
                    for gi, (r0, nr) in enumerate(rest):
                        psw = _W if nr == 8 else _WO
                        ps = pspool.tile([128, nr, psw], f32, tag="ps")
                        for tap in range(_TAPS):
                            mm(ps, img_tiles, ct, r0, nr, tap,
                               start=(tap == 0), stop=(tap == _TAPS - 1))
                        finish_group(
                            ps, img, ct, r0, nr,
                            pipelined_tail=(
                                img == _BPC - 1 and ct == 1 and gi == len(rest) - 1
                            ),
                        )

    nc.compile()
    _nc_cache = nc
    return nc


def _prep_in_maps(x, weight):
    x = np.ascontiguousarray(np.asarray(x), dtype=np.float32)
    w = np.ascontiguousarray(np.asarray(weight), dtype=np.float32)
    assert x.shape == (_B, _CIN, _H, _W), x.shape
    assert w.shape == (_COUT, _CIN, 3, 3), w.shape
    # w[ci, dy*3+dx, co] = weight[co, ci, dy, dx]
    wt = np.ascontiguousarray(w.transpose(1, 2, 3, 0).reshape(_CIN, _TAPS, _COUT))
    xs = x.reshape(_NCORES, _BPC, _CIN, _H, _W)
    return [{"x": np.ascontiguousarray(xs[i]), "w": wt} for i in range(_NCORES)]


def _run(x, weight, **kwargs):
    from concourse.bass_utils import run_bass_kernel_spmd

    nc = _build()
    res = run_bass_kernel_spmd(
        nc, _prep_in_maps(x, weight), core_ids=list(range(_NCORES)), **kwargs
    )
    out = np.concatenate([r["o"] for r in res.results], axis=0)
    return out.astype(np.float32, copy=False), res


def kernel(x, weight):
    out, _ = _run(x, weight)
    return out



# revision 3
# speedup vs baseline: 1.0470x; 1.0470x over previous
"""Trainium2 Bass kernel for batched 3x3 VALID conv (NCHW / OIHW).

x: [32, 128, 64, 64] f32, weight: [256, 128, 3, 3] f32 -> out: [32, 256, 62, 62] f32.

Strategy: data-parallel over batch across 8 NeuronCores (4 images each).
Inputs are cast to bf16 on the host (PE rate is identical to fp32r, but
input DMA bytes halve; PSUM accumulation stays fp32 so the error is
~0.4% << the 2e-2 gate). Per core the conv is 9 shift-matmuls
accumulated in PSUM over 62-wide strided windows (no garbage columns):
  out[co, y, x] += W[dy,dx][ci,co].T @ x[ci, y+dy, x+dx]
Output rows are split into 8 groups (7x8 + 1x6 rows); each group's
[co, nr, 62] accumulator fills one PSUM bank. Groups run in halves of
4 with a tap-outer loop (one weight load per tap serving 4 matmuls),
so 4 banks accumulate while the previous 4 drain through the vector
engine to SBUF and out via alternating sync/gpsimd DMA queues.
The first half-block runs group-major so the PE starts as soon as
tap 0 plus ten image rows have landed, consuming weight taps in DMA
arrival order off the (fastest-starting) sync queue.
"""

import numpy as np

_B, _CIN, _H, _W = 32, 128, 64, 64
_COUT = 256
_HO, _WO = 62, 62
_NCORES = 8
_BPC = _B // _NCORES  # images per core
_TAPS = 9

_GROUPS_A = [(0, 8), (8, 8), (16, 8), (24, 8)]
_GROUPS_B = [(32, 8), (40, 8), (48, 8), (56, 6)]

_nc_cache = None


def _build():
    global _nc_cache
    if _nc_cache is not None:
        return _nc_cache

    import concourse.bass as bass
    import concourse.mybir as mybir
    from concourse import bacc
    from concourse.tile import TileContext

    f32 = mybir.dt.float32
    bf16 = mybir.dt.bfloat16

    nc = bacc.Bacc("TRN2", target_bir_lowering=False)
    x_d = nc.dram_tensor("x", [_BPC, _CIN, _H, _W], bf16, kind="ExternalInput")
    w_d = nc.dram_tensor("w", [_CIN, _TAPS, _COUT], bf16, kind="ExternalInput")
    o_d = nc.dram_tensor("o", [_BPC, _COUT, _HO, _WO], f32, kind="ExternalOutput")

    with TileContext(nc) as tc:
        with (
            tc.tile_pool(name="wpool", bufs=1) as wpool,
            tc.tile_pool(name="xpool", bufs=2) as xpool,
            tc.tile_pool(name="spool", bufs=6) as spool,
            tc.tile_pool(name="pspool", bufs=8, space=bass.MemorySpace.PSUM) as pspool,
        ):
            w_sb = wpool.tile([_CIN, _TAPS, _COUT], bf16)
            x_tile_a = xpool.tile([_CIN, _H, _W], bf16, tag="x")
            x_tile_b = xpool.tile([_CIN, _H, _W], bf16, tag="x")
            x_tiles = [x_tile_a, x_tile_b]

            # PE warmup on a zeroed bf16 tile: keeps the HAM clock ramping
            # while the head DMAs stream in. Small (128-row) matmuls so the
            # first real matmul can slot in with minimal delay.
            wup = wpool.tile([128, 128], bf16)
            wps = pspool.tile([128, 512], f32, tag="ps")
            nc.vector.memset(wup[:], 0)
            for _ in range(20):
                nc.tensor.matmul(wps[:, 0:128], wup[:], wup[:], start=True, stop=True)

            # Head DMAs. The sync queue spins up first, so it carries the
            # full weight tensor (per-tap pieces, in consumption order)
            # interleaved with the first image rows; scalar and gpsimd pick
            # up the remaining img0 rows. Image prefetches ride behind on
            # the scalar queue only, so their WAR waits never block output
            # stores (which rotate sync/gpsimd).
            nc.sync.dma_start(w_sb[:, 0:1, :], w_d[:, 0:1, :])
            nc.sync.dma_start(x_tiles[0][:, 0:10, :], x_d[0, :, 0:10, :])
            nc.sync.dma_start(w_sb[:, 1:2, :], w_d[:, 1:2, :])
            nc.sync.dma_start(w_sb[:, 2:3, :], w_d[:, 2:3, :])
            nc.sync.dma_start(w_sb[:, 3:4, :], w_d[:, 3:4, :])
            nc.sync.dma_start(x_tiles[0][:, 10:18, :], x_d[0, :, 10:18, :])
            nc.sync.dma_start(w_sb[:, 4:5, :], w_d[:, 4:5, :])
            nc.sync.dma_start(w_sb[:, 5:6, :], w_d[:, 5:6, :])
            nc.sync.dma_start(w_sb[:, 6:7, :], w_d[:, 6:7, :])
            nc.sync.dma_start(w_sb[:, 7:8, :], w_d[:, 7:8, :])
            nc.sync.dma_start(w_sb[:, 8:9, :], w_d[:, 8:9, :])
            nc.scalar.dma_start(x_tiles[0][:, 18:30, :], x_d[0, :, 18:30, :])
            nc.scalar.dma_start(x_tiles[0][:, 30:40, :], x_d[0, :, 30:40, :])
            nc.gpsimd.dma_start(x_tiles[0][:, 40:52, :], x_d[0, :, 40:52, :])
            nc.sync.dma_start(x_tiles[0][:, 52:64, :], x_d[0, :, 52:64, :])
            # img1 prefetch (into the b tile) behind img0's scalar chunks.
            nc.scalar.dma_start(x_tiles[1][:], x_d[1])

            store_ctr = [0]

            def mm(ps, x_sb, ct, r0, nr, tap, start, stop):
                dy, dx = divmod(tap, 3)
                nc.tensor.matmul(
                    ps[:, 0:nr, :],
                    w_sb[:, tap, ct * 128 : (ct + 1) * 128],
                    x_sb[:, r0 + dy : r0 + dy + nr, dx : dx + _WO],
                    start=start,
                    stop=stop,
                )

            def drain(ps, img, ct, r0, nr, copy_eng, st_queue):
                st = spool.tile([128, nr, _WO], f32, tag="st")
                o_slice = o_d[img, ct * 128 : (ct + 1) * 128, r0 : r0 + nr, :]
                copy_eng(st[:], ps[:, 0:nr, :])
                st_queue.dma_start(o_slice, st[:])

            for img in range(_BPC):
                x_sb = x_tiles[img % 2]
                for ct in range(_COUT // 128):
                    # Prefetch image img+1 early (img1 already issued above).
                    if ct == 0 and 2 <= img + 1 < _BPC:
                        nc.scalar.dma_start(x_tiles[(img + 1) % 2][:], x_d[img + 1])
                    for half, groups in enumerate((_GROUPS_A, _GROUPS_B)):
                        ps_l = [
                            pspool.tile([128, nr, _WO], f32, tag="ps", name="ps")
                            for (r0, nr) in groups
                        ]
                        if img == 0 and ct == 0 and half == 0:
                            # Head: group-major so the PE starts on tap 0 as
                            # soon as it and the first rows land, consuming
                            # taps in sync-queue arrival order.
                            for ps, (r0, nr) in zip(ps_l, groups):
                                for tap in range(_TAPS):
                                    mm(ps, x_sb, ct, r0, nr, tap,
                                       start=(tap == 0), stop=(tap == _TAPS - 1))
                        else:
                            # Steady state: tap-outer, one weight load feeds
                            # four matmuls into four PSUM banks.
                            for tap in range(_TAPS):
                                for ps, (r0, nr) in zip(ps_l, groups):
                                    mm(ps, x_sb, ct, r0, nr, tap,
                                       start=(tap == 0), stop=(tap == _TAPS - 1))
                        last_half = img == _BPC - 1 and ct == 1 and half == 1
                        for gi, (ps, (r0, nr)) in enumerate(zip(ps_l, groups)):
                            if last_half:
                                # Tail: copies split across vector+scalar,
                                # stores across sync+gpsimd, to drain the
                                # final four banks in parallel.
                                copy_eng = (
                                    nc.vector.tensor_copy
                                    if gi % 2 == 0
                                    else nc.scalar.copy
                                )
                                q = nc.sync if gi % 2 == 0 else nc.gpsimd
                                drain(ps, img, ct, r0, nr, copy_eng, q)
                            else:
                                q = nc.sync if store_ctr[0] % 2 == 0 else nc.gpsimd
                                store_ctr[0] += 1
                                drain(ps, img, ct, r0, nr, nc.vector.tensor_copy, q)

    nc.compile()
    _nc_cache = nc
    return nc


def _prep_in_maps(x, weight):
    from concourse import mybir

    np_bf16 = mybir.dt.np(mybir.dt.bfloat16)
    x = np.asarray(x, dtype=np.float32)
    w = np.asarray(weight, dtype=np.float32)
    assert x.shape == (_B, _CIN, _H, _W), x.shape
    assert w.shape == (_COUT, _CIN, 3, 3), w.shape
    # w[ci, dy*3+dx, co] = weight[co, ci, dy, dx]
    wt = np.ascontiguousarray(
        w.transpose(1, 2, 3, 0).reshape(_CIN, _TAPS, _COUT)
    ).astype(np_bf16)
    xs = x.reshape(_NCORES, _BPC, _CIN, _H, _W).astype(np_bf16)
    return [{"x": np.ascontiguousarray(xs[i]), "w": wt} for i in range(_NCORES)]


def _run(x, weight, **kwargs):
    from concourse.bass_utils import run_bass_kernel_spmd

    nc = _build()
    res = run_bass_kernel_spmd(
        nc, _prep_in_maps(x, weight), core_ids=list(range(_NCORES)), **kwargs
    )
    out = np.concatenate([r["o"] for r in res.results], axis=0)
    return out.astype(np.float32, copy=False), res


def kernel(x, weight):
    out, _ = _run(x, weight)
    return out


# revision 5
# speedup vs baseline: 1.0719x; 1.0238x over previous
"""Trainium2 Bass kernel for batched 3x3 VALID conv (NCHW / OIHW).

x: [32, 128, 64, 64] f32, weight: [256, 128, 3, 3] f32 -> out: [32, 256, 62, 62] f32.

Strategy: data-parallel over batch across 8 NeuronCores (4 images each).
Inputs are cast to bf16 on the host (PE rate is identical to fp32r, but
input DMA bytes halve; PSUM accumulation stays fp32 so the error is
~0.4% << the 2e-2 gate). Per core the conv is 9 shift-matmuls
accumulated in PSUM over 62-wide strided windows (no garbage columns):
  out[co, y, x] += W[dy,dx][ci,co].T @ x[ci, y+dy, x+dx]
Output rows are split into 8 groups (7x8 + 1x6 rows); each group's
[co, nr, 62] accumulator fills one PSUM bank. Groups run in halves of
4 with a tap-outer loop (one weight load per tap serving 4 matmuls),
so 4 banks accumulate while the previous 4 drain through the vector
engine to SBUF and out via alternating sync/gpsimd DMA queues.
The first half-block runs group-major so the PE starts as soon as
tap 0 plus ten image rows have landed, consuming weight taps in DMA
arrival order off the (fastest-starting) sync queue.
"""

import numpy as np

_B, _CIN, _H, _W = 32, 128, 64, 64
_COUT = 256
_HO, _WO = 62, 62
_NCORES = 8
_BPC = _B // _NCORES  # images per core
_TAPS = 9

_GROUPS_A = [(0, 8), (8, 8), (16, 8), (24, 8)]
_GROUPS_B = [(32, 8), (40, 8), (48, 8), (56, 6)]

_nc_cache = None


def _build():
    global _nc_cache
    if _nc_cache is not None:
        return _nc_cache

    import concourse.bass as bass
    import concourse.mybir as mybir
    from concourse import bacc
    from concourse.tile import TileContext

    f32 = mybir.dt.float32
    bf16 = mybir.dt.bfloat16

    nc = bacc.Bacc("TRN2", target_bir_lowering=False)
    x_d = nc.dram_tensor("x", [_BPC, _CIN, _H, _W], bf16, kind="ExternalInput")
    w_d = nc.dram_tensor("w", [_CIN, _TAPS, _COUT], bf16, kind="ExternalInput")
    o_d = nc.dram_tensor("o", [_BPC, _COUT, _HO, _WO], f32, kind="ExternalOutput")

    with TileContext(nc) as tc:
        with (
            tc.tile_pool(name="wpool", bufs=1) as wpool,
            tc.tile_pool(name="xpool", bufs=2) as xpool,
            tc.tile_pool(name="spool", bufs=6) as spool,
            tc.tile_pool(name="pspool", bufs=8, space=bass.MemorySpace.PSUM) as pspool,
        ):
            w_sb = wpool.tile([_CIN, _TAPS, _COUT], bf16)
            x_tile_a = xpool.tile([_CIN, _H, _W], bf16, tag="x")
            x_tile_b = xpool.tile([_CIN, _H, _W], bf16, tag="x")
            x_tiles = [x_tile_a, x_tile_b]

            # PE warmup on a zeroed bf16 tile: keeps the HAM clock ramping
            # while the head DMAs stream in. Small (128-row) matmuls so the
            # first real matmul can slot in with minimal delay.
            wup = wpool.tile([128, 128], bf16)
            wps = pspool.tile([128, 512], f32, tag="ps")
            nc.vector.memset(wup[:], 0)
            for _ in range(20):
                nc.tensor.matmul(wps[:, 0:128], wup[:], wup[:], start=True, stop=True)

            # Head DMAs. DMA throughput depends on per-partition line size,
            # so few big chunks beat many small ones. The sync queue spins
            # up first and carries the weights (two chunks so tap 0 lands
            # early); scalar + gpsimd split img0's rows. Image prefetches
            # ride behind on the scalar queue only, so their WAR waits never
            # block output stores (which rotate sync/gpsimd).
            nc.sync.dma_start(w_sb[:, 0:4, :], w_d[:, 0:4, :])
            nc.sync.dma_start(w_sb[:, 4:9, :], w_d[:, 4:9, :])
            nc.scalar.dma_start(x_tiles[0][:, 0:16, :], x_d[0, :, 0:16, :])
            nc.scalar.dma_start(x_tiles[0][:, 16:34, :], x_d[0, :, 16:34, :])
            nc.gpsimd.dma_start(x_tiles[0][:, 34:64, :], x_d[0, :, 34:64, :])
            # img1 prefetch (into the b tile) behind img0's scalar chunks.
            nc.scalar.dma_start(x_tiles[1][:], x_d[1])

            store_ctr = [0]

            def mm(ps, x_sb, ct, r0, nr, tap, start, stop):
                dy, dx = divmod(tap, 3)
                nc.tensor.matmul(
                    ps[:, 0:nr, :],
                    w_sb[:, tap, ct * 128 : (ct + 1) * 128],
                    x_sb[:, r0 + dy : r0 + dy + nr, dx : dx + _WO],
                    start=start,
                    stop=stop,
                )

            def drain(ps, img, ct, r0, nr, copy_eng, st_queue):
                st = spool.tile([128, nr, _WO], f32, tag="st")
                o_slice = o_d[img, ct * 128 : (ct + 1) * 128, r0 : r0 + nr, :]
                copy_eng(st[:], ps[:, 0:nr, :])
                st_queue.dma_start(o_slice, st[:])

            for img in range(_BPC):
                x_sb = x_tiles[img % 2]
                for ct in range(_COUT // 128):
                    # Prefetch image img+1 early (img1 already issued above).
                    if ct == 0 and 2 <= img + 1 < _BPC:
                        nc.scalar.dma_start(x_tiles[(img + 1) % 2][:], x_d[img + 1])
                    for half, groups in enumerate((_GROUPS_A, _GROUPS_B)):
                        ps_l = [
                            pspool.tile([128, nr, _WO], f32, tag="ps", name="ps")
                            for (r0, nr) in groups
                        ]
                        if img == 0 and ct == 0 and half == 0:
                            # Head: group-major so the PE starts on tap 0 as
                            # soon as it and the first rows land, consuming
                            # taps in sync-queue arrival order.
                            for ps, (r0, nr) in zip(ps_l, groups):
                                for tap in range(_TAPS):
                                    mm(ps, x_sb, ct, r0, nr, tap,
                                       start=(tap == 0), stop=(tap == _TAPS - 1))
                        else:
                            # Steady state: tap-outer, one weight load feeds
                            # four matmuls into four PSUM banks.
                            for tap in range(_TAPS):
                                for ps, (r0, nr) in zip(ps_l, groups):
                                    mm(ps, x_sb, ct, r0, nr, tap,
                                       start=(tap == 0), stop=(tap == _TAPS - 1))
                        last_half = img == _BPC - 1 and ct == 1 and half == 1
                        late = img == _BPC - 1 and ct == 1
                        for gi, (ps, (r0, nr)) in enumerate(zip(ps_l, groups)):
                            if last_half:
                                # Tail: copies split across vector+scalar,
                                # stores across sync+scalar (the gpsimd
                                # software queue drains too slowly), so the
                                # final four banks empty in parallel.
                                copy_eng = (
                                    nc.vector.tensor_copy
                                    if gi % 2 == 0
                                    else nc.scalar.copy
                                )
                                q = nc.sync if gi % 2 == 0 else nc.scalar
                                drain(ps, img, ct, r0, nr, copy_eng, q)
                            elif late:
                                drain(ps, img, ct, r0, nr, nc.vector.tensor_copy, nc.sync)
                            else:
                                q = nc.sync if store_ctr[0] % 2 == 0 else nc.gpsimd
                                store_ctr[0] += 1
                                drain(ps, img, ct, r0, nr, nc.vector.tensor_copy, q)

    nc.compile()
    _nc_cache = nc
    return nc


def _prep_in_maps(x, weight):
    from concourse import mybir

    np_bf16 = mybir.dt.np(mybir.dt.bfloat16)
    x = np.asarray(x, dtype=np.float32)
    w = np.asarray(weight, dtype=np.float32)
    assert x.shape == (_B, _CIN, _H, _W), x.shape
    assert w.shape == (_COUT, _CIN, 3, 3), w.shape
    # w[ci, dy*3+dx, co] = weight[co, ci, dy, dx]
    wt = np.ascontiguousarray(
        w.transpose(1, 2, 3, 0).reshape(_CIN, _TAPS, _COUT)
    ).astype(np_bf16)
    xs = x.reshape(_NCORES, _BPC, _CIN, _H, _W).astype(np_bf16)
    return [{"x": np.ascontiguousarray(xs[i]), "w": wt} for i in range(_NCORES)]


def _run(x, weight, **kwargs):
    from concourse.bass_utils import run_bass_kernel_spmd

    nc = _build()
    res = run_bass_kernel_spmd(
        nc, _prep_in_maps(x, weight), core_ids=list(range(_NCORES)), **kwargs
    )
    out = np.concatenate([r["o"] for r in res.results], axis=0)
    return out.astype(np.float32, copy=False), res


def kernel(x, weight):
    out, _ = _run(x, weight)
    return out


# revision 8
# speedup vs baseline: 1.0737x; 1.0017x over previous
"""Trainium2 Bass kernel for batched 3x3 VALID conv (NCHW / OIHW).

x: [32, 128, 64, 64] f32, weight: [256, 128, 3, 3] f32 -> out: [32, 256, 62, 62] f32.

Strategy: data-parallel over batch across 8 NeuronCores (4 images each).
Inputs are cast to bf16 on the host (PE rate is identical to fp32r, but
input DMA bytes halve; PSUM accumulation stays fp32 so the error is
~0.4% << the 2e-2 gate). Per core the conv is 9 shift-matmuls
accumulated in PSUM over 62-wide strided windows (no garbage columns):
  out[co, y, x] += W[dy,dx][ci,co].T @ x[ci, y+dy, x+dx]
Output rows are split into 8 groups (7x8 + 1x6 rows); each group's
[co, nr, 62] accumulator fills one PSUM bank. Groups run in halves of
4 with a tap-outer loop (one weight load per tap serving 4 matmuls),
so 4 banks accumulate while the previous 4 drain through the vector
engine to SBUF and out via alternating sync/gpsimd DMA queues.
The first half-block runs group-major so the PE starts as soon as
tap 0 plus ten image rows have landed, consuming weight taps in DMA
arrival order off the (fastest-starting) sync queue.
"""

import numpy as np

_B, _CIN, _H, _W = 32, 128, 64, 64
_COUT = 256
_HO, _WO = 62, 62
_NCORES = 8
_BPC = _B // _NCORES  # images per core
_TAPS = 9

_GROUPS_A = [(0, 8), (8, 8), (16, 8), (24, 8)]
_GROUPS_B = [(32, 8), (40, 8), (48, 8), (56, 6)]

_nc_cache = None


def _build():
    global _nc_cache
    if _nc_cache is not None:
        return _nc_cache

    import concourse.bass as bass
    import concourse.mybir as mybir
    from concourse import bacc
    from concourse.tile import TileContext

    f32 = mybir.dt.float32
    bf16 = mybir.dt.bfloat16

    nc = bacc.Bacc("TRN2", target_bir_lowering=False)
    x_d = nc.dram_tensor("x", [_BPC, _CIN, _H, _W], bf16, kind="ExternalInput")
    w_d = nc.dram_tensor("w", [_CIN, _TAPS, _COUT], bf16, kind="ExternalInput")
    o_d = nc.dram_tensor("o", [_BPC, _COUT, _HO, _WO], f32, kind="ExternalOutput")

    with TileContext(nc) as tc:
        with (
            tc.tile_pool(name="wpool", bufs=1) as wpool,
            tc.tile_pool(name="xpool", bufs=2) as xpool,
            tc.tile_pool(name="spool", bufs=6) as spool,
            tc.tile_pool(name="pspool", bufs=8, space=bass.MemorySpace.PSUM) as pspool,
        ):
            w_sb = wpool.tile([_CIN, _TAPS, _COUT], bf16)
            x_tile_a = xpool.tile([_CIN, _H, _W], bf16, tag="x")
            x_tile_b = xpool.tile([_CIN, _H, _W], bf16, tag="x")
            x_tiles = [x_tile_a, x_tile_b]

            # PE warmup on a zeroed bf16 tile: keeps the PE continuously
            # busy from the end of the prologue so the HAM clock (which
            # also gates DMA throughput) ramps to full speed before the
            # real matmuls and bulk DMA traffic need it. Big 512-row
            # matmuls, sized to end about when the head DMAs land.
            wup = wpool.tile([128, 512], bf16)
            wps = pspool.tile([128, 512], f32, tag="ps")
            nc.vector.memset(wup[:], 0)
            for _ in range(8):
                nc.tensor.matmul(wps[:], wup[:, 0:128], wup[:], start=True, stop=True)

            # Head DMAs. DMA throughput depends on per-partition line size,
            # so few big chunks beat many small ones. The sync queue spins
            # up first and carries the weights (two chunks so tap 0 lands
            # early); scalar + gpsimd split img0's rows. Image prefetches
            # ride behind on the scalar queue only, so their WAR waits never
            # block output stores (which rotate sync/gpsimd).
            nc.sync.dma_start(w_sb[:, 0:4, :], w_d[:, 0:4, :])
            nc.sync.dma_start(w_sb[:, 4:9, :], w_d[:, 4:9, :])
            nc.scalar.dma_start(x_tiles[0][:, 0:16, :], x_d[0, :, 0:16, :])
            nc.scalar.dma_start(x_tiles[0][:, 16:34, :], x_d[0, :, 16:34, :])
            nc.gpsimd.dma_start(x_tiles[0][:, 34:64, :], x_d[0, :, 34:64, :])
            # img1 prefetch (into the b tile) behind img0's scalar chunks.
            nc.scalar.dma_start(x_tiles[1][:], x_d[1])

            store_ctr = [0]

            def mm(ps, x_sb, ct, r0, nr, tap, start, stop):
                dy, dx = divmod(tap, 3)
                nc.tensor.matmul(
                    ps[:, 0:nr, :],
                    w_sb[:, tap, ct * 128 : (ct + 1) * 128],
                    x_sb[:, r0 + dy : r0 + dy + nr, dx : dx + _WO],
                    start=start,
                    stop=stop,
                )

            def drain(ps, img, ct, r0, nr, copy_eng, st_queue):
                st = spool.tile([128, nr, _WO], f32, tag="st")
                o_slice = o_d[img, ct * 128 : (ct + 1) * 128, r0 : r0 + nr, :]
                copy_eng(st[:], ps[:, 0:nr, :])
                st_queue.dma_start(o_slice, st[:])

            for img in range(_BPC):
                x_sb = x_tiles[img % 2]
                for ct in range(_COUT // 128):
                    # Prefetch image img+1 early (img1 already issued above).
                    if ct == 0 and 2 <= img + 1 < _BPC:
                        nc.scalar.dma_start(x_tiles[(img + 1) % 2][:], x_d[img + 1])
                    for half, groups in enumerate((_GROUPS_A, _GROUPS_B)):
                        ps_l = [
                            pspool.tile([128, nr, _WO], f32, tag="ps", name="ps")
                            for (r0, nr) in groups
                        ]
                        head_half = img == 0 and ct == 0 and half == 0
                        tail_half = img == _BPC - 1 and ct == 1 and half == 1
                        if head_half or tail_half:
                            # Head: group-major so the PE starts on tap 0 as
                            # soon as it and the first rows land. Tail:
                            # group-major so the first three groups finish
                            # (and drain) while the last still accumulates.
                            for ps, (r0, nr) in zip(ps_l, groups):
                                for tap in range(_TAPS):
                                    mm(ps, x_sb, ct, r0, nr, tap,
                                       start=(tap == 0), stop=(tap == _TAPS - 1))
                        else:
                            # Steady state: tap-outer, one weight load feeds
                            # four matmuls into four PSUM banks.
                            for tap in range(_TAPS):
                                for ps, (r0, nr) in zip(ps_l, groups):
                                    mm(ps, x_sb, ct, r0, nr, tap,
                                       start=(tap == 0), stop=(tap == _TAPS - 1))
                        late = img == _BPC - 1 and ct == 1
                        for gi, (ps, (r0, nr)) in enumerate(zip(ps_l, groups)):
                            if tail_half and gi == len(groups) - 1:
                                # Final group: halve the copy across
                                # vector+scalar and the store across
                                # sync+scalar so the very last bytes drain
                                # through two engines and two queues.
                                st = spool.tile([128, nr, _WO], f32, tag="st")
                                o_sl = o_d[img, ct * 128 : (ct + 1) * 128, r0 : r0 + nr, :]
                                h = nr // 2
                                nc.vector.tensor_copy(st[:, 0:h, :], ps[:, 0:h, :])
                                nc.scalar.copy(st[:, h:nr, :], ps[:, h:nr, :])
                                nc.sync.dma_start(o_sl[:, 0:h, :], st[:, 0:h, :])
                                nc.scalar.dma_start(o_sl[:, h:nr, :], st[:, h:nr, :])
                            elif tail_half:
                                # Earlier tail groups finish 1.9us apart
                                # (group-major): drain them under the
                                # remaining matmuls, off the slow gpsimd
                                # queue.
                                copy_eng = (
                                    nc.vector.tensor_copy
                                    if gi % 2 == 0
                                    else nc.scalar.copy
                                )
                                q = nc.sync if gi % 2 == 0 else nc.scalar
                                drain(ps, img, ct, r0, nr, copy_eng, q)
                            elif late:
                                drain(ps, img, ct, r0, nr, nc.vector.tensor_copy, nc.sync)
                            else:
                                q = nc.sync if store_ctr[0] % 2 == 0 else nc.gpsimd
                                store_ctr[0] += 1
                                drain(ps, img, ct, r0, nr, nc.vector.tensor_copy, q)

    nc.compile()
    _nc_cache = nc
    return nc


def _prep_in_maps(x, weight):
    from concourse import mybir

    np_bf16 = mybir.dt.np(mybir.dt.bfloat16)
    x = np.asarray(x, dtype=np.float32)
    w = np.asarray(weight, dtype=np.float32)
    assert x.shape == (_B, _CIN, _H, _W), x.shape
    assert w.shape == (_COUT, _CIN, 3, 3), w.shape
    # w[ci, dy*3+dx, co] = weight[co, ci, dy, dx]
    wt = np.ascontiguousarray(
        w.transpose(1, 2, 3, 0).reshape(_CIN, _TAPS, _COUT)
    ).astype(np_bf16)
    xs = x.reshape(_NCORES, _BPC, _CIN, _H, _W).astype(np_bf16)
    return [{"x": np.ascontiguousarray(xs[i]), "w": wt} for i in range(_NCORES)]


def _run(x, weight, **kwargs):
    from concourse.bass_utils import run_bass_kernel_spmd

    nc = _build()
    res = run_bass_kernel_spmd(
        nc, _prep_in_maps(x, weight), core_ids=list(range(_NCORES)), **kwargs
    )
    out = np.concatenate([r["o"] for r in res.results], axis=0)
    return out.astype(np.float32, copy=False), res


def kernel(x, weight):
    out, _ = _run(x, weight)
    return out


# revision 10
# speedup vs baseline: 1.1114x; 1.0351x over previous
"""Trainium2 Bass kernel for batched 3x3 VALID conv (NCHW / OIHW).

x: [32, 128, 64, 64] f32, weight: [256, 128, 3, 3] f32 -> out: [32, 256, 62, 62] f32.

Strategy: data-parallel over batch across 8 NeuronCores (4 images each).
Inputs are cast to bf16 on the host (PE rate is identical to fp32r, but
input DMA bytes halve; PSUM accumulation stays fp32 so the error is
~0.4% << the 2e-2 gate). Per core the conv is 9 shift-matmuls
accumulated in PSUM over 62-wide strided windows (no garbage columns):
  out[co, y, x] += W[dy,dx][ci,co].T @ x[ci, y+dy, x+dx]
Output rows are split into 8 groups (7x8 + 1x6 rows); each group's
[co, nr, 62] accumulator fills one PSUM bank. Groups run in halves of
4 with a tap-outer loop (one weight load per tap serving 4 matmuls),
so 4 banks accumulate while the previous 4 drain through the vector
engine to SBUF and out via alternating sync/gpsimd DMA queues.
The first half-block runs group-major so the PE starts as soon as
tap 0 plus ten image rows have landed, consuming weight taps in DMA
arrival order off the (fastest-starting) sync queue.
"""

import numpy as np

_B, _CIN, _H, _W = 32, 128, 64, 64
_COUT = 256
_HO, _WO = 62, 62
_NCORES = 8
_BPC = _B // _NCORES  # images per core
_TAPS = 9

_GROUPS_A = [(0, 8), (8, 8), (16, 8), (24, 8)]
_GROUPS_B = [(32, 8), (40, 8), (48, 8), (56, 6)]

_nc_cache = None


def _build():
    global _nc_cache
    if _nc_cache is not None:
        return _nc_cache

    import concourse.bass as bass
    import concourse.mybir as mybir
    from concourse import bacc
    from concourse.tile import TileContext

    f32 = mybir.dt.float32
    bf16 = mybir.dt.bfloat16

    nc = bacc.Bacc("TRN2", target_bir_lowering=False)
    x_d = nc.dram_tensor("x", [_BPC, _CIN, _H, _W], bf16, kind="ExternalInput")
    w_d = nc.dram_tensor("w", [_CIN, _TAPS, _COUT], bf16, kind="ExternalInput")
    o_d = nc.dram_tensor("o", [_BPC, _COUT, _HO, _WO], f32, kind="ExternalOutput")

    with TileContext(nc) as tc:
        with (
            tc.tile_pool(name="wpool", bufs=1) as wpool,
            tc.tile_pool(name="xpool", bufs=2) as xpool,
            tc.tile_pool(name="spool", bufs=6) as spool,
            tc.tile_pool(name="pspool", bufs=8, space=bass.MemorySpace.PSUM) as pspool,
        ):
            w_sb = wpool.tile([_CIN, _TAPS, _COUT], bf16)
            x_tile_a = xpool.tile([_CIN, _H, _W], bf16, tag="x")
            x_tile_b = xpool.tile([_CIN, _H, _W], bf16, tag="x")
            x_tiles = [x_tile_a, x_tile_b]

            # PE warmup on a zeroed bf16 tile: keeps the PE continuously
            # busy from the end of the prologue so the HAM clock (which
            # also gates DMA throughput) ramps to full speed before the
            # real matmuls and bulk DMA traffic need it. Big 512-row
            # matmuls, sized to end about when the head DMAs land.
            wup = wpool.tile([128, 512], bf16)
            wps = pspool.tile([128, 512], f32, tag="ps")
            nc.vector.memset(wup[:], 0)
            for _ in range(8):
                nc.tensor.matmul(wps[:], wup[:, 0:128], wup[:], start=True, stop=True)
            # Tapering tail of small warmups: keeps the PE busy (no HAM
            # down-throttle) while the head DMAs finish, with fine enough
            # granularity that the first real matmul slots in quickly.
            for _ in range(16):
                nc.tensor.matmul(
                    wps[:, 0:128], wup[:, 0:128], wup[:, 0:128], start=True, stop=True
                )

            # Head DMAs. DMA throughput depends on per-partition line size,
            # so few big chunks beat many small ones. The sync queue spins
            # up first and carries the weights (two chunks so tap 0 lands
            # early); scalar + gpsimd split img0's rows. Image prefetches
            # ride behind on the scalar queue only, so their WAR waits never
            # block output stores (which rotate sync/gpsimd).
            nc.sync.dma_start(w_sb[:, 0:4, :], w_d[:, 0:4, :])
            nc.sync.dma_start(w_sb[:, 4:9, :], w_d[:, 4:9, :])
            nc.scalar.dma_start(x_tiles[0][:, 0:10, :], x_d[0, :, 0:10, :])
            nc.scalar.dma_start(x_tiles[0][:, 10:22, :], x_d[0, :, 10:22, :])
            nc.scalar.dma_start(x_tiles[0][:, 22:34, :], x_d[0, :, 22:34, :])
            nc.gpsimd.dma_start(x_tiles[0][:, 34:64, :], x_d[0, :, 34:64, :])
            # img1 prefetch (into the b tile) behind img0's scalar chunks.
            nc.scalar.dma_start(x_tiles[1][:], x_d[1])

            store_ctr = [0]

            def mm(ps, x_sb, ct, r0, nr, tap, start, stop):
                dy, dx = divmod(tap, 3)
                nc.tensor.matmul(
                    ps[:, 0:nr, :],
                    w_sb[:, tap, ct * 128 : (ct + 1) * 128],
                    x_sb[:, r0 + dy : r0 + dy + nr, dx : dx + _WO],
                    start=start,
                    stop=stop,
                )

            def drain(ps, img, ct, r0, nr, copy_eng, st_queue):
                st = spool.tile([128, nr, _WO], f32, tag="st")
                o_slice = o_d[img, ct * 128 : (ct + 1) * 128, r0 : r0 + nr, :]
                copy_eng(st[:], ps[:, 0:nr, :])
                st_queue.dma_start(o_slice, st[:])

            for img in range(_BPC):
                x_sb = x_tiles[img % 2]
                for ct in range(_COUT // 128):
                    # Prefetch image img+1 early (img1 already issued above).
                    if ct == 0 and 2 <= img + 1 < _BPC:
                        nc.scalar.dma_start(x_tiles[(img + 1) % 2][:], x_d[img + 1])
                    for half, groups in enumerate((_GROUPS_A, _GROUPS_B)):
                        ps_l = [
                            pspool.tile([128, nr, _WO], f32, tag="ps", name="ps")
                            for (r0, nr) in groups
                        ]
                        head_half = img == 0 and ct == 0 and half == 0
                        tail_half = img == _BPC - 1 and ct == 1 and half == 1
                        if head_half or tail_half:
                            # Head: group-major so the PE starts on tap 0 as
                            # soon as it and the first rows land. Tail:
                            # group-major so the first three groups finish
                            # (and drain) while the last still accumulates.
                            for ps, (r0, nr) in zip(ps_l, groups):
                                for tap in range(_TAPS):
                                    mm(ps, x_sb, ct, r0, nr, tap,
                                       start=(tap == 0), stop=(tap == _TAPS - 1))
                        else:
                            # Steady state: tap-outer, one weight load feeds
                            # four matmuls into four PSUM banks.
                            for tap in range(_TAPS):
                                for ps, (r0, nr) in zip(ps_l, groups):
                                    mm(ps, x_sb, ct, r0, nr, tap,
                                       start=(tap == 0), stop=(tap == _TAPS - 1))
                        late = img == _BPC - 1 and ct == 1
                        for gi, (ps, (r0, nr)) in enumerate(zip(ps_l, groups)):
                            if tail_half and gi == len(groups) - 1:
                                # Final group: halve the copy across
                                # vector+scalar and the store across
                                # sync+scalar so the very last bytes drain
                                # through two engines and two queues.
                                st = spool.tile([128, nr, _WO], f32, tag="st")
                                o_sl = o_d[img, ct * 128 : (ct + 1) * 128, r0 : r0 + nr, :]
                                h = nr // 2
                                nc.vector.tensor_copy(st[:, 0:h, :], ps[:, 0:h, :])
                                nc.scalar.copy(st[:, h:nr, :], ps[:, h:nr, :])
                                nc.sync.dma_start(o_sl[:, 0:h, :], st[:, 0:h, :])
                                nc.scalar.dma_start(o_sl[:, h:nr, :], st[:, h:nr, :])
                            elif tail_half:
                                # Earlier tail groups finish 1.9us apart
                                # (group-major): drain them under the
                                # remaining matmuls, off the slow gpsimd
                                # queue.
                                copy_eng = (
                                    nc.vector.tensor_copy
                                    if gi % 2 == 0
                                    else nc.scalar.copy
                                )
                                q = nc.sync if gi % 2 == 0 else nc.scalar
                                drain(ps, img, ct, r0, nr, copy_eng, q)
                            elif late:
                                drain(ps, img, ct, r0, nr, nc.vector.tensor_copy, nc.sync)
                            else:
                                q = nc.sync if store_ctr[0] % 2 == 0 else nc.gpsimd
                                store_ctr[0] += 1
                                drain(ps, img, ct, r0, nr, nc.vector.tensor_copy, q)

    nc.compile()
    _nc_cache = nc
    return nc


def _prep_in_maps(x, weight):
    from concourse import mybir

    np_bf16 = mybir.dt.np(mybir.dt.bfloat16)
    x = np.asarray(x, dtype=np.float32)
    w = np.asarray(weight, dtype=np.float32)
    assert x.shape == (_B, _CIN, _H, _W), x.shape
    assert w.shape == (_COUT, _CIN, 3, 3), w.shape
    # w[ci, dy*3+dx, co] = weight[co, ci, dy, dx]
    wt = np.ascontiguousarray(
        w.transpose(1, 2, 3, 0).reshape(_CIN, _TAPS, _COUT)
    ).astype(np_bf16)
    xs = x.reshape(_NCORES, _BPC, _CIN, _H, _W).astype(np_bf16)
    return [{"x": np.ascontiguousarray(xs[i]), "w": wt} for i in range(_NCORES)]


def _run(x, weight, **kwargs):
    from concourse.bass_utils import run_bass_kernel_spmd

    nc = _build()
    res = run_bass_kernel_spmd(
        nc, _prep_in_maps(x, weight), core_ids=list(range(_NCORES)), **kwargs
    )
    out = np.concatenate([r["o"] for r in res.results], axis=0)
    return out.astype(np.float32, copy=False), res


def kernel(x, weight):
    out, _ = _run(x, weight)
    return out


# revision 12
# speedup vs baseline: 1.1156x; 1.0038x over previous
"""Trainium2 Bass kernel for batched 3x3 VALID conv (NCHW / OIHW).

x: [32, 128, 64, 64] f32, weight: [256, 128, 3, 3] f32 -> out: [32, 256, 62, 62] f32.

Strategy: data-parallel over batch across 8 NeuronCores (4 images each).
Inputs are cast to bf16 on the host (PE rate is identical to fp32r, but
input DMA bytes halve; PSUM accumulation stays fp32 so the error is
~0.4% << the 2e-2 gate). Per core the conv is 9 shift-matmuls
accumulated in PSUM over 62-wide strided windows (no garbage columns):
  out[co, y, x] += W[dy,dx][ci,co].T @ x[ci, y+dy, x+dx]
Output rows are split into 8 groups (7x8 + 1x6 rows); each group's
[co, nr, 62] accumulator fills one PSUM bank. Groups run in halves of
4 with a tap-outer loop (one weight load per tap serving 4 matmuls),
so 4 banks accumulate while the previous 4 drain through the vector
engine to SBUF and out via alternating sync/gpsimd DMA queues.
The first half-block runs group-major so the PE starts as soon as
tap 0 plus ten image rows have landed, consuming weight taps in DMA
arrival order off the (fastest-starting) sync queue.
"""

import numpy as np

_B, _CIN, _H, _W = 32, 128, 64, 64
_COUT = 256
_HO, _WO = 62, 62
_NCORES = 8
_BPC = _B // _NCORES  # images per core
_TAPS = 9

_GROUPS_A = [(0, 8), (8, 8), (16, 8), (24, 8)]
_GROUPS_B = [(32, 8), (40, 8), (48, 8), (56, 6)]

_nc_cache = None


def _build():
    global _nc_cache
    if _nc_cache is not None:
        return _nc_cache

    import concourse.bass as bass
    import concourse.mybir as mybir
    from concourse import bacc
    from concourse.tile import TileContext

    f32 = mybir.dt.float32
    bf16 = mybir.dt.bfloat16

    nc = bacc.Bacc("TRN2", target_bir_lowering=False)
    x_d = nc.dram_tensor("x", [_BPC, _CIN, _H, _W], bf16, kind="ExternalInput")
    w_d = nc.dram_tensor("w", [_CIN, _TAPS, _COUT], bf16, kind="ExternalInput")
    o_d = nc.dram_tensor("o", [_BPC, _COUT, _HO, _WO], f32, kind="ExternalOutput")

    with TileContext(nc) as tc:
        with (
            tc.tile_pool(name="wpool", bufs=1) as wpool,
            tc.tile_pool(name="xpool", bufs=2) as xpool,
            tc.tile_pool(name="spool", bufs=6) as spool,
            tc.tile_pool(name="pspool", bufs=8, space=bass.MemorySpace.PSUM) as pspool,
        ):
            w_sb = wpool.tile([_CIN, _TAPS, _COUT], bf16)
            x_tile_a = xpool.tile([_CIN, _H, _W], bf16, tag="x")
            x_tile_b = xpool.tile([_CIN, _H, _W], bf16, tag="x")
            x_tiles = [x_tile_a, x_tile_b]

            # PE warmup on a zeroed bf16 tile: keeps the PE continuously
            # busy from the end of the prologue so the HAM clock (which
            # also gates DMA throughput) ramps to full speed before the
            # real matmuls and bulk DMA traffic need it. Big 512-row
            # matmuls, sized to end about when the head DMAs land.
            wup = wpool.tile([128, 512], bf16)
            wps = pspool.tile([128, 512], f32, tag="ps")
            nc.vector.memset(wup[:], 0)
            for _ in range(8):
                nc.tensor.matmul(wps[:], wup[:, 0:128], wup[:], start=True, stop=True)
            # Tapering tail of small warmups: keeps the PE busy (no HAM
            # down-throttle) while the head DMAs finish, with fine enough
            # granularity that the first real matmul slots in quickly.
            for _ in range(16):
                nc.tensor.matmul(
                    wps[:, 0:128], wup[:, 0:128], wup[:, 0:128], start=True, stop=True
                )

            # Head DMAs. DMA throughput depends on per-partition line size,
            # so few big chunks beat many small ones. The sync queue spins
            # up first and carries the weights (two chunks so tap 0 lands
            # early); scalar + gpsimd split img0's rows. Image prefetches
            # ride behind on the scalar queue only, so their WAR waits never
            # block output stores (which rotate sync/gpsimd).
            nc.sync.dma_start(w_sb[:, 0:4, :], w_d[:, 0:4, :])
            nc.gpsimd.dma_start(w_sb[:, 4:9, :], w_d[:, 4:9, :])
            nc.scalar.dma_start(x_tiles[0][:, 0:10, :], x_d[0, :, 0:10, :])
            nc.scalar.dma_start(x_tiles[0][:, 10:22, :], x_d[0, :, 10:22, :])
            nc.scalar.dma_start(x_tiles[0][:, 22:34, :], x_d[0, :, 22:34, :])
            nc.sync.dma_start(x_tiles[0][:, 34:50, :], x_d[0, :, 34:50, :])
            nc.gpsimd.dma_start(x_tiles[0][:, 50:64, :], x_d[0, :, 50:64, :])
            # img1 prefetch (into the b tile) behind img0's scalar chunks.
            nc.scalar.dma_start(x_tiles[1][:], x_d[1])

            store_ctr = [0]

            def mm(ps, x_sb, ct, r0, nr, tap, start, stop):
                dy, dx = divmod(tap, 3)
                nc.tensor.matmul(
                    ps[:, 0:nr, :],
                    w_sb[:, tap, ct * 128 : (ct + 1) * 128],
                    x_sb[:, r0 + dy : r0 + dy + nr, dx : dx + _WO],
                    start=start,
                    stop=stop,
                )

            def drain(ps, img, ct, r0, nr, copy_eng, st_queue):
                st = spool.tile([128, nr, _WO], f32, tag="st")
                o_slice = o_d[img, ct * 128 : (ct + 1) * 128, r0 : r0 + nr, :]
                copy_eng(st[:], ps[:, 0:nr, :])
                st_queue.dma_start(o_slice, st[:])

            for img in range(_BPC):
                x_sb = x_tiles[img % 2]
                for ct in range(_COUT // 128):
                    # Prefetch image img+1 early (img1 already issued above).
                    if ct == 0 and 2 <= img + 1 < _BPC:
                        nc.scalar.dma_start(x_tiles[(img + 1) % 2][:], x_d[img + 1])
                    for half, groups in enumerate((_GROUPS_A, _GROUPS_B)):
                        ps_l = [
                            pspool.tile([128, nr, _WO], f32, tag="ps", name="ps")
                            for (r0, nr) in groups
                        ]
                        head_half = img == 0 and ct == 0 and half == 0
                        tail_half = img == _BPC - 1 and ct == 1 and half == 1
                        if head_half or tail_half:
                            # Head: group-major so the PE starts on tap 0 as
                            # soon as it and the first rows land. Tail:
                            # group-major so the first three groups finish
                            # (and drain) while the last still accumulates.
                            for ps, (r0, nr) in zip(ps_l, groups):
                                for tap in range(_TAPS):
                                    mm(ps, x_sb, ct, r0, nr, tap,
                                       start=(tap == 0), stop=(tap == _TAPS - 1))
                        else:
                            # Steady state: tap-outer, one weight load feeds
                            # four matmuls into four PSUM banks.
                            for tap in range(_TAPS):
                                for ps, (r0, nr) in zip(ps_l, groups):
                                    mm(ps, x_sb, ct, r0, nr, tap,
                                       start=(tap == 0), stop=(tap == _TAPS - 1))
                        late = img == _BPC - 1 and ct == 1
                        for gi, (ps, (r0, nr)) in enumerate(zip(ps_l, groups)):
                            if tail_half and gi >= len(groups) - 2:
                                # Final two groups: halve the copy across
                                # vector+scalar and the store across
                                # sync+scalar so the very last bytes drain
                                # through two engines and two queues.
                                st = spool.tile([128, nr, _WO], f32, tag="st")
                                o_sl = o_d[img, ct * 128 : (ct + 1) * 128, r0 : r0 + nr, :]
                                h = nr // 2
                                nc.vector.tensor_copy(st[:, 0:h, :], ps[:, 0:h, :])
                                nc.scalar.copy(st[:, h:nr, :], ps[:, h:nr, :])
                                nc.sync.dma_start(o_sl[:, 0:h, :], st[:, 0:h, :])
                                nc.scalar.dma_start(o_sl[:, h:nr, :], st[:, h:nr, :])
                            elif tail_half:
                                # Earlier tail groups finish 1.9us apart
                                # (group-major): drain them under the
                                # remaining matmuls, off the slow gpsimd
                                # queue.
                                copy_eng = (
                                    nc.vector.tensor_copy
                                    if gi % 2 == 0
                                    else nc.scalar.copy
                                )
                                q = nc.sync if gi % 2 == 0 else nc.scalar
                                drain(ps, img, ct, r0, nr, copy_eng, q)
                            elif late:
                                drain(ps, img, ct, r0, nr, nc.vector.tensor_copy, nc.sync)
                            else:
                                q = nc.sync if store_ctr[0] % 2 == 0 else nc.gpsimd
                                store_ctr[0] += 1
                                drain(ps, img, ct, r0, nr, nc.vector.tensor_copy, q)

    nc.compile()
    _nc_cache = nc
    return nc


def _prep_in_maps(x, weight):
    from concourse import mybir

    np_bf16 = mybir.dt.np(mybir.dt.bfloat16)
    x = np.asarray(x, dtype=np.float32)
    w = np.asarray(weight, dtype=np.float32)
    assert x.shape == (_B, _CIN, _H, _W), x.shape
    assert w.shape == (_COUT, _CIN, 3, 3), w.shape
    # w[ci, dy*3+dx, co] = weight[co, ci, dy, dx]
    wt = np.ascontiguousarray(
        w.transpose(1, 2, 3, 0).reshape(_CIN, _TAPS, _COUT)
    ).astype(np_bf16)
    xs = x.reshape(_NCORES, _BPC, _CIN, _H, _W).astype(np_bf16)
    return [{"x": np.ascontiguousarray(xs[i]), "w": wt} for i in range(_NCORES)]


def _run(x, weight, **kwargs):
    from concourse.bass_utils import run_bass_kernel_spmd

    nc = _build()
    res = run_bass_kernel_spmd(
        nc, _prep_in_maps(x, weight), core_ids=list(range(_NCORES)), **kwargs
    )
    out = np.concatenate([r["o"] for r in res.results], axis=0)
    return out.astype(np.float32, copy=False), res


def kernel(x, weight):
    out, _ = _run(x, weight)
    return out


# revision 13
# speedup vs baseline: 1.1159x; 1.0002x over previous
"""Trainium2 Bass kernel for batched 3x3 VALID conv (NCHW / OIHW).

x: [32, 128, 64, 64] f32, weight: [256, 128, 3, 3] f32 -> out: [32, 256, 62, 62] f32.

Strategy: data-parallel over batch across 8 NeuronCores (4 images each).
Inputs are cast to bf16 on the host (PE rate is identical to fp32r, but
input DMA bytes halve; PSUM accumulation stays fp32 so the error is
~0.4% << the 2e-2 gate). Per core the conv is 9 shift-matmuls
accumulated in PSUM over 62-wide strided windows (no garbage columns):
  out[co, y, x] += W[dy,dx][ci,co].T @ x[ci, y+dy, x+dx]
Output rows are split into 8 groups (7x8 + 1x6 rows); each group's
[co, nr, 62] accumulator fills one PSUM bank. Groups run in halves of
4 with a tap-outer loop (one weight load per tap serving 4 matmuls),
so 4 banks accumulate while the previous 4 drain through the vector
engine to SBUF and out via alternating sync/gpsimd DMA queues.
The first half-block runs group-major so the PE starts as soon as
tap 0 plus ten image rows have landed, consuming weight taps in DMA
arrival order off the (fastest-starting) sync queue.
"""

import numpy as np

_B, _CIN, _H, _W = 32, 128, 64, 64
_COUT = 256
_HO, _WO = 62, 62
_NCORES = 8
_BPC = _B // _NCORES  # images per core
_TAPS = 9

_GROUPS_A = [(0, 8), (8, 8), (16, 8), (24, 8)]
_GROUPS_B = [(32, 8), (40, 8), (48, 8), (56, 6)]

_nc_cache = None


def _build():
    global _nc_cache
    if _nc_cache is not None:
        return _nc_cache

    import concourse.bass as bass
    import concourse.mybir as mybir
    from concourse import bacc
    from concourse.tile import TileContext

    f32 = mybir.dt.float32
    bf16 = mybir.dt.bfloat16

    nc = bacc.Bacc("TRN2", target_bir_lowering=False)
    x_d = nc.dram_tensor("x", [_BPC, _CIN, _H, _W], bf16, kind="ExternalInput")
    w_d = nc.dram_tensor("w", [_CIN, _TAPS, _COUT], bf16, kind="ExternalInput")
    o_d = nc.dram_tensor("o", [_BPC, _COUT, _HO, _WO], f32, kind="ExternalOutput")

    with TileContext(nc) as tc:
        with (
            tc.tile_pool(name="wpool", bufs=1) as wpool,
            tc.tile_pool(name="xpool", bufs=2) as xpool,
            tc.tile_pool(name="spool", bufs=6) as spool,
            tc.tile_pool(name="pspool", bufs=8, space=bass.MemorySpace.PSUM) as pspool,
        ):
            w_sb = wpool.tile([_CIN, _TAPS, _COUT], bf16)
            x_tile_a = xpool.tile([_CIN, _H, _W], bf16, tag="x")
            x_tile_b = xpool.tile([_CIN, _H, _W], bf16, tag="x")
            x_tiles = [x_tile_a, x_tile_b]

            # PE warmup on a zeroed bf16 tile: keeps the PE continuously
            # busy from the end of the prologue so the HAM clock (which
            # also gates DMA throughput) ramps to full speed before the
            # real matmuls and bulk DMA traffic need it. Big 512-row
            # matmuls, sized to end about when the head DMAs land.
            wup = wpool.tile([128, 512], bf16)
            wps = pspool.tile([128, 512], f32, tag="ps")
            dummy = wpool.tile([128, 512], bf16)
            nc.vector.memset(wup[:], 0)
            # Extra element-wise busy-work on the vector engine alongside
            # the PE warmups: more engine activity pushes the HAM power
            # ramp (which also gates DMA throughput) to full speed sooner.
            for _ in range(12):
                nc.vector.tensor_copy(dummy[:], wup[:])
            for _ in range(8):
                nc.tensor.matmul(wps[:], wup[:, 0:128], wup[:], start=True, stop=True)
            # Tapering tail of small warmups: keeps the PE busy (no HAM
            # down-throttle) while the head DMAs finish, with fine enough
            # granularity that the first real matmul slots in quickly.
            for _ in range(16):
                nc.tensor.matmul(
                    wps[:, 0:128], wup[:, 0:128], wup[:, 0:128], start=True, stop=True
                )

            # Head DMAs. DMA throughput depends on per-partition line size,
            # so few big chunks beat many small ones. The sync queue spins
            # up first and carries the weights (two chunks so tap 0 lands
            # early); scalar + gpsimd split img0's rows. Image prefetches
            # ride behind on the scalar queue only, so their WAR waits never
            # block output stores (which rotate sync/gpsimd).
            nc.sync.dma_start(w_sb[:, 0:4, :], w_d[:, 0:4, :])
            nc.gpsimd.dma_start(w_sb[:, 4:9, :], w_d[:, 4:9, :])
            nc.scalar.dma_start(x_tiles[0][:, 0:10, :], x_d[0, :, 0:10, :])
            nc.scalar.dma_start(x_tiles[0][:, 10:22, :], x_d[0, :, 10:22, :])
            nc.scalar.dma_start(x_tiles[0][:, 22:34, :], x_d[0, :, 22:34, :])
            nc.sync.dma_start(x_tiles[0][:, 34:50, :], x_d[0, :, 34:50, :])
            nc.gpsimd.dma_start(x_tiles[0][:, 50:64, :], x_d[0, :, 50:64, :])
            # img1 prefetch (into the b tile) behind img0's scalar chunks.
            nc.scalar.dma_start(x_tiles[1][:], x_d[1])

            store_ctr = [0]

            def mm(ps, x_sb, ct, r0, nr, tap, start, stop):
                dy, dx = divmod(tap, 3)
                nc.tensor.matmul(
                    ps[:, 0:nr, :],
                    w_sb[:, tap, ct * 128 : (ct + 1) * 128],
                    x_sb[:, r0 + dy : r0 + dy + nr, dx : dx + _WO],
                    start=start,
                    stop=stop,
                )

            def drain(ps, img, ct, r0, nr, copy_eng, st_queue):
                st = spool.tile([128, nr, _WO], f32, tag="st")
                o_slice = o_d[img, ct * 128 : (ct + 1) * 128, r0 : r0 + nr, :]
                copy_eng(st[:], ps[:, 0:nr, :])
                st_queue.dma_start(o_slice, st[:])

            for img in range(_BPC):
                x_sb = x_tiles[img % 2]
                for ct in range(_COUT // 128):
                    # Prefetch image img+1 early (img1 already issued above).
                    if ct == 0 and 2 <= img + 1 < _BPC:
                        nc.scalar.dma_start(x_tiles[(img + 1) % 2][:], x_d[img + 1])
                    for half, groups in enumerate((_GROUPS_A, _GROUPS_B)):
                        ps_l = [
                            pspool.tile([128, nr, _WO], f32, tag="ps", name="ps")
                            for (r0, nr) in groups
                        ]
                        head_half = img == 0 and ct == 0 and half == 0
                        tail_half = img == _BPC - 1 and ct == 1 and half == 1
                        if head_half or tail_half:
                            # Head: group-major so the PE starts on tap 0 as
                            # soon as it and the first rows land. Tail:
                            # group-major so the first three groups finish
                            # (and drain) while the last still accumulates.
                            for ps, (r0, nr) in zip(ps_l, groups):
                                for tap in range(_TAPS):
                                    mm(ps, x_sb, ct, r0, nr, tap,
                                       start=(tap == 0), stop=(tap == _TAPS - 1))
                        else:
                            # Steady state: tap-outer, one weight load feeds
                            # four matmuls into four PSUM banks.
                            for tap in range(_TAPS):
                                for ps, (r0, nr) in zip(ps_l, groups):
                                    mm(ps, x_sb, ct, r0, nr, tap,
                                       start=(tap == 0), stop=(tap == _TAPS - 1))
                        late = img == _BPC - 1 and ct == 1
                        for gi, (ps, (r0, nr)) in enumerate(zip(ps_l, groups)):
                            if tail_half and gi >= len(groups) - 2:
                                # Final two groups: halve the copy across
                                # vector+scalar and the store across
                                # sync+scalar so the very last bytes drain
                                # through two engines and two queues.
                                st = spool.tile([128, nr, _WO], f32, tag="st")
                                o_sl = o_d[img, ct * 128 : (ct + 1) * 128, r0 : r0 + nr, :]
                                h = nr // 2
                                nc.vector.tensor_copy(st[:, 0:h, :], ps[:, 0:h, :])
                                nc.scalar.copy(st[:, h:nr, :], ps[:, h:nr, :])
                                nc.sync.dma_start(o_sl[:, 0:h, :], st[:, 0:h, :])
                                nc.scalar.dma_start(o_sl[:, h:nr, :], st[:, h:nr, :])
                            elif tail_half:
                                # Earlier tail groups finish 1.9us apart
                                # (group-major): drain them under the
                                # remaining matmuls, off the slow gpsimd
                                # queue.
                                copy_eng = (
                                    nc.vector.tensor_copy
                                    if gi % 2 == 0
                                    else nc.scalar.copy
                                )
                                q = nc.sync if gi % 2 == 0 else nc.scalar
                                drain(ps, img, ct, r0, nr, copy_eng, q)
                            elif late:
                                drain(ps, img, ct, r0, nr, nc.vector.tensor_copy, nc.sync)
                            else:
                                q = nc.sync if store_ctr[0] % 2 == 0 else nc.gpsimd
                                store_ctr[0] += 1
                                drain(ps, img, ct, r0, nr, nc.vector.tensor_copy, q)

    nc.compile()
    _nc_cache = nc
    return nc


def _prep_in_maps(x, weight):
    from concourse import mybir

    np_bf16 = mybir.dt.np(mybir.dt.bfloat16)
    x = np.asarray(x, dtype=np.float32)
    w = np.asarray(weight, dtype=np.float32)
    assert x.shape == (_B, _CIN, _H, _W), x.shape
    assert w.shape == (_COUT, _CIN, 3, 3), w.shape
    # w[ci, dy*3+dx, co] = weight[co, ci, dy, dx]
    wt = np.ascontiguousarray(
        w.transpose(1, 2, 3, 0).reshape(_CIN, _TAPS, _COUT)
    ).astype(np_bf16)
    xs = x.reshape(_NCORES, _BPC, _CIN, _H, _W).astype(np_bf16)
    return [{"x": np.ascontiguousarray(xs[i]), "w": wt} for i in range(_NCORES)]


def _run(x, weight, **kwargs):
    from concourse.bass_utils import run_bass_kernel_spmd

    nc = _build()
    res = run_bass_kernel_spmd(
        nc, _prep_in_maps(x, weight), core_ids=list(range(_NCORES)), **kwargs
    )
    out = np.concatenate([r["o"] for r in res.results], axis=0)
    return out.astype(np.float32, copy=False), res


def kernel(x, weight):
    out, _ = _run(x, weight)
    return out
